# revision 44
# baseline (speedup 1.0000x reference)
"""Trainium2 Bass kernel for nn_BoundaryModule_38422777430159.

Reference computation (B=4, C=256, T=256, N=10, D=40, DIM0=512, DIM1=128):
  x1 = sample(feature)            # (B,C,N,D,T) via (T, N*D*T) smp matmul
  x2 = leaky(einsum('bcndt,ocn->bodt', x1, w0) + b0)
  x3 = leaky(w1 @ x2 + b1)        # 1x1 conv
  x4 = leaky(conv3x3(x3, w2) + b2)
  out = sigmoid(w3 @ x4 + b3)     # (B, D, T)

Device strategy (8 NeuronCores, SPMD; core i handles b = i//2 and
t-half th = i%2 with a 1-column halo). Nearly the whole pipeline runs
in fp8-e4m3 with fp32 PSUM accumulation:
  A      = feature.T @ w0 per n (PE fp32r, w0 pre-divided by the fp8
           scale sa host-side), stored as one fp8 tile [128, 20, 512]
  x2     = sampling contraction vs the (2560, 5200) W slice, fp8
           DoubleRow matmuls (each slot contracts a PAIR of 128-row
           chunks at 2 rows/cycle); W streamed from HBM in fp8
  x3     = 1x1 conv as 2 DoubleRow matmuls (o-chunk pairs)
  conv   = 3 DoubleRow (dy 0+1 tap pairs) + 3 plain fp8 (dy 2) matmuls
  out    = bf16 1x1 + sigmoid
All bias+leaky(+rescale) steps run on the Scalar (ACT) engine via the
Lrelu activation with per-partition scale/bias inputs; the fp8 scales
are folded into those inputs host-side, so the DVE only does the conv
pad copies. PSUM is always fp32; scale bounds are loose upper bounds
(fp8's 2^17 dynamic range absorbs the slack).
"""
import os
import sys

for _p in ("/opt/trn_rl_repo", "/root/.axon_site/_ro/trn_rl_repo"):
    if os.path.isdir(_p) and _p not in sys.path:
        sys.path.append(_p)

import numpy as np
import ml_dtypes

import concourse.bass as bass
import concourse.tile as tile
from concourse import mybir
from concourse.bass_utils import run_bass_kernel_spmd
from concourse.tile_rust import add_dep_helper

T = 256
N = 10
D = 40
B = 4
C_IN = 256
DIM0 = 512
DIM1 = 128

TW = 130          # t-window incl. 1-col halo each side
TWP = 144         # padded conv row stride (mult of 16 for fp8 DR APs)
COLS = D * TW     # 5200 matmul columns per core
FW = 400          # free-dim chunk (<=512 psum bank)
NF = COLS // FW   # 13
K = 2 * N         # 20 contraction chunks of 128 (tau-chunk major within n)
DCH = 3           # conv d-rows per psum group
NDCH = (D + DCH - 1) // DCH  # 14 (13*3 + 1)

F32 = mybir.dt.float32
F32R = mybir.dt.float32r
BF16 = mybir.dt.bfloat16
F8 = mybir.dt.float8e4
DR = mybir.MatmulPerfMode.DoubleRow
E4NP = ml_dtypes.float8_e4m3
BFNP = ml_dtypes.bfloat16
Q8 = 240.0        # e4m3 max finite
LRELU = mybir.ActivationFunctionType.Lrelu


def _legalize_waits(nc, limit=1):
    """This walrus build allows a single embedded sync wait per real
    instruction; move the excess onto standalone NoOp wait-carriers."""
    moved = 0
    for f in nc.m.functions:
        for bb in f.blocks:
            il = bb.instructions
            out = []
            changed = False
            for inst in il:
                si = inst.sync_info
                ty = type(inst).__name__
                if (si and si.on_wait and len(si.on_wait) > limit
                        and ty not in ("InstEventSemaphore", "InstNoOp")):
                    keep = si.on_wait[-limit:]
                    for w in si.on_wait[:-limit]:
                        out.append(mybir.InstNoOp(
                            name=f"waitnop-{nc.next_id()}",
                            sync_info=mybir.SyncInfo(on_wait=[w], on_update=[]),
                            bass_nofuse=True,
                            engine=inst.engine,
                        ))
                        moved += 1
                    inst.sync_info = mybir.SyncInfo(
                        on_wait=keep, on_update=si.on_update)
                    changed = True
                out.append(inst)
            if changed:
                bb.instructions = out
    return moved


def _build_program(pairs):
    """pairs: tuple per f-chunk of (ka, kb) index pairs; slot (f, j)
    contracts A chunks ka and kb (256 rows) against a host-packed fp8
    W pair tile via a DoubleRow matmul."""
    nslots = [len(p) for p in pairs]
    stot = sum(nslots)
    nc = bass.Bass(trn_type="TRN2")
    MAX = mybir.AluOpType.max
    MULT = mybir.AluOpType.mult
    ADD = mybir.AluOpType.add

    feat_d = nc.dram_tensor("feat", [128, 2, T], F8, kind="ExternalInput")
    w0_d = nc.dram_tensor("w0t", [N, 128, 2, DIM0], F8, kind="ExternalInput")
    wsmp_d = nc.dram_tensor("wsmp", [stot, 128, 2, FW], F8,
                            kind="ExternalInput")
    w1_d = nc.dram_tensor("w1q", [2, 128, 2, DIM1], F8, kind="ExternalInput")
    w2p_d = nc.dram_tensor("w2qp", [3, 128, 2, DIM1], F8,
                           kind="ExternalInput")
    w2s_d = nc.dram_tensor("w2qs", [3, 128, DIM1], F8, kind="ExternalInput")
    w3_d = nc.dram_tensor("w3t", [DIM1, 1], BF16, kind="ExternalInput")
    b0_d = nc.dram_tensor("b0x", [4, 128, 1], F32, kind="ExternalInput")
    b1_d = nc.dram_tensor("b1c", [128, 1], F32, kind="ExternalInput")
    b2_d = nc.dram_tensor("b2d", [128, 1], F32, kind="ExternalInput")
    b3_d = nc.dram_tensor("b3", [1, 1], F32, kind="ExternalInput")
    sc_d = nc.dram_tensor("scl", [4, 128, 1], F32, kind="ExternalInput")
    out_d = nc.dram_tensor("out", [1, D * TW], F32, kind="ExternalOutput")

    with tile.TileContext(nc) as tc:
        with (
            tc.tile_pool(name="inp", bufs=1) as inp,
            tc.tile_pool(name="wst", bufs=24) as wst,
            tc.tile_pool(name="apool", bufs=1) as apool,
            tc.tile_pool(name="x2p", bufs=2) as x2p,
            tc.tile_pool(name="x3p", bufs=1) as x3p,
            tc.tile_pool(name="x4p", bufs=2) as x4p,
            tc.tile_pool(name="scr", bufs=2) as scr,
            tc.tile_pool(name="outp", bufs=1) as outp,
            tc.tile_pool(name="psb", bufs=1, space="PSUM") as psb,
            tc.tile_pool(name="psg", bufs=2, space="PSUM") as psg,
        ):
            # ---- input DMAs (all destinations write-once) ----
            # big streams split across the two HWDGE rings (SP + ACT)
            feat_t = inp.tile([128, 2, T], F8, tag="feat", name="feat_sb")
            nc.sync.dma_start(feat_t[:], feat_d[:])
            # interleave w0 and first-chunk W-pair DMAs in CREATION order —
            # each HWDGE ring drains in program order, so this is what
            # actually trickle-starts stages A and B together
            w0t = [inp.tile([128, 2, DIM0], F8, tag=f"w0_{n}", name=f"w0_{n}")
                   for n in range(N)]
            wpre = {j: wst.tile([128, 2, FW], F8, tag="w", name=f"wt_0_{j}")
                    for j in range(nslots[0])}
            for i in range(max(N, nslots[0])):
                if i < N:
                    eng = nc.sync if i % 2 == 0 else nc.scalar
                    eng.dma_start(w0t[i][:], w0_d[i])
                if i < nslots[0]:
                    eng = nc.scalar if i % 2 == 0 else nc.sync
                    eng.dma_start(wpre[i][:], wsmp_d[i])
            w1qt = []
            for p in range(2):
                t_ = inp.tile([128, 2, DIM1], F8, tag=f"w1_{p}",
                              name=f"w1q_{p}")
                nc.scalar.dma_start(t_[:], w1_d[p])
                w1qt.append(t_)
            w2qp = []
            for dx in range(3):
                t_ = inp.tile([128, 2, DIM1], F8, tag=f"w2p_{dx}",
                              name=f"w2qp_{dx}")
                nc.scalar.dma_start(t_[:], w2p_d[dx])
                w2qp.append(t_)
            w2qs = []
            for dx in range(3):
                t_ = inp.tile([128, DIM1], F8, tag=f"w2s_{dx}",
                              name=f"w2qs_{dx}")
                nc.scalar.dma_start(t_[:], w2s_d[dx])
                w2qs.append(t_)
            w3t = inp.tile([128, 1], BF16, tag="w3", name="w3t_sb")
            nc.scalar.dma_start(w3t[:], w3_d[:])
            b0t = inp.tile([128, 4], F32, tag="b0", name="b0_sb")
            nc.sync.dma_start(b0t[:].rearrange("p (a b) -> p a b", b=1),
                              b0_d[:].transpose((1, 0, 2)))
            b1t = inp.tile([128, 1], F32, tag="b1", name="b1_sb")
            nc.sync.dma_start(b1t[:], b1_d[:])
            b2t = inp.tile([128, 1], F32, tag="b2", name="b2_sb")
            nc.sync.dma_start(b2t[:], b2_d[:])
            b3t = inp.tile([1, 1], F32, tag="b3", name="b3_sb")
            nc.sync.dma_start(b3t[:], b3_d[:])
            sct = inp.tile([128, 4], F32, tag="scl", name="scl_sb")
            nc.sync.dma_start(sct[:].rearrange("p (a b) -> p a b", b=1),
                              sc_d[:].transpose((1, 0, 2)))
            cx = sct[:, 0:1]
            cc = sct[:, 1:2]
            cd = sct[:, 2:3]
            ca = sct[:, 3:4]

            # conv pad buffer: only the border cells the 3x3 taps read need
            # zeroing (rows 0/41, cols 0/131); interior is overwritten and
            # cols >= 132 are never read
            pad = x3p.tile([128, D + 2, TWP], F8, tag="pad", name="padbuf")
            nc.vector.memset(pad[:, 0:1, 0:TW + 2], 0.0)
            nc.vector.memset(pad[:, D + 1:D + 2, 0:TW + 2], 0.0)
            nc.vector.memset(pad[:, :, 0:1], 0.0)
            nc.vector.memset(pad[:, :, TW + 1:TW + 2], 0.0)

            # ---- teach engines the input-DMA ticks (1 wait per inst) ----
            dve_scr = scr.tile([128, 4], F32, tag="dscr", name="dve_scr")
            nc.vector.tensor_copy(dve_scr[:, 0:1], b1t[:])
            nc.vector.tensor_copy(dve_scr[:, 1:2], b2t[:])
            nc.vector.tensor_copy(dve_scr[:, 2:3], b0t[:, 0:1])
            nc.scalar.mul(dve_scr[0:1, 3:4], b3t[:], 1.0)
            # warm-up accumulation group on the fp32r tiles, spread so stage
            # A can start as soon as the tiles it needs have landed
            warm = psg.tile([1, 4], F32, tag="g", name="warm_ps")

            def warm_mm(t_, first, last):
                nc.tensor.matmul(warm[:], t_[:, 0:1, 0:1], t_[:, 0:1, 0:4],
                                 start=first, stop=last)

            warm_mm(feat_t, True, False)

            # ---- stage A: A[k] = (feature chunk).T @ w0_n -> fp8 [tau,o] --
            # one fp8 DoubleRow matmul per k (c-chunk pair); 4-way PSUM
            # rotation with evictions alternating DVE / ACT. The first
            # f-chunk's g0 sampling matmuls are FUSED into this loop (on the
            # psg "d" psum buffers, which are free until the conv) so the PE
            # FIFO stays dense through the ramp.
            a8 = apool.tile([128, K, DIM0], F8, tag="a8", name="a8_sb")
            pf0 = pairs[0]
            yt0 = x2p.tile([128, 4, FW], F8, tag="x2", name="x2_0")
            f0a0 = psg.tile([128, FW], F32, tag="d", name="psb0_0")
            f0a1 = psg.tile([128, FW], F32, tag="d", name="psb0_1")
            jp = 0

            def emit_f0_g0(j):
                ka, kb = pf0[j]
                for o, acc in ((0, f0a0), (1, f0a1)):
                    nc.tensor.matmul(
                        acc[:],
                        a8[:, ka:kb + 1:(kb - ka), o * 128:(o + 1) * 128],
                        wpre[j][:],
                        start=(j == 0), stop=(j == len(pf0) - 1),
                        perf_mode=DR,
                    )

            for n in range(N):
                warm_mm(w0t[n], False, n == N - 1)
                for tch in range(2):
                    k = n * 2 + tch
                    ps = psb.tile([128, DIM0], F32, tag=f"b{k % 4}",
                                  name=f"psa{n}_{tch}")
                    nc.tensor.matmul(
                        ps[:],
                        feat_t[:, 0:2, tch * 128:(tch + 1) * 128],
                        w0t[n][:],
                        start=True, stop=True,
                        perf_mode=DR,
                    )
                    if k % 2 == 0:
                        nc.vector.tensor_scalar_mul(a8[:, k, :], ps[:], ca)
                    else:
                        nc.scalar.activation(
                            a8[:, k, :], ps[:],
                            mybir.ActivationFunctionType.Copy, scale=ca)
                    while jp < len(pf0) and pf0[jp][1] <= k - 2:
                        emit_f0_g0(jp)
                        jp += 1
            while jp < len(pf0):
                emit_f0_g0(jp)
                jp += 1

            # ---- stages B (fp8 DR sampling) + C (1x1, fp8 DR) per f-chunk -
            # och pairs double-buffered in PSUM so consecutive f-chunks
            # overlap
            x3 = x3p.tile([128, COLS], F8, tag="x3", name="x3_sb")
            x3g = x3[:].rearrange("p (d t) -> p d t", d=D)
            sbase = 0
            next_pad = 0
            wts_by_f = {0: dict(wpre)}

            def issue_w_dmas(f, fbase):
                d = {}
                for j in range(len(pairs[f])):
                    wt = wst.tile([128, 2, FW], F8, tag="w",
                                  name=f"wt_{f}_{j}")
                    eng = nc.scalar if j % 3 == 2 else nc.sync
                    eng.dma_start(wt[:], wsmp_d[fbase + j])
                    d[j] = wt
                wts_by_f[f] = d

            for f in range(NF):
                pf = pairs[f]
                # W tiles for f+1 issue now, one full f-period ahead
                if f + 1 < NF:
                    issue_w_dmas(f + 1, sbase + len(pf))
                wts = wts_by_f.pop(f)
                if f == 0:
                    yt = yt0
                else:
                    yt = x2p.tile([128, 4, FW], F8, tag="x2", name=f"x2_{f}")
                for g in range(2):
                    if f == 0 and g == 0:
                        a0, a1 = f0a0, f0a1   # matmuls fused into stage A
                    else:
                        t0_, t1_ = ((0, 1) if f == 0 else (2 * g, 2 * g + 1))
                        a0 = psb.tile([128, FW], F32, tag=f"b{t0_}",
                                      name=f"psb{f}_{2 * g}")
                        a1 = psb.tile([128, FW], F32, tag=f"b{t1_}",
                                      name=f"psb{f}_{2 * g + 1}")
                        for j, (ka, kb) in enumerate(pf):
                            wt = wts[j]
                            for o, acc in ((2 * g, a0), (2 * g + 1, a1)):
                                nc.tensor.matmul(
                                    acc[:],
                                    a8[:, ka:kb + 1:(kb - ka),
                                       o * 128:(o + 1) * 128],
                                    wt[:],
                                    start=(j == 0), stop=(j == len(pf) - 1),
                                    perf_mode=DR,
                                )
                    for o, acc in ((2 * g, a0), (2 * g + 1, a1)):
                        if o % 2 == 0 or (f >= 10 and o == 1):
                            nc.scalar.activation(
                                yt[:, o, :], acc[:], LRELU,
                                bias=b0t[:, o:o + 1], scale=cx, alpha=0.01)
                        else:
                            tmp = scr.tile([128, FW], F32, tag="t2",
                                           name=f"t2_{f}_{o}")
                            nc.vector.tensor_scalar(
                                tmp[:], acc[:], cx, b0t[:, o:o + 1],
                                MULT, ADD)
                            nc.vector.scalar_tensor_tensor(
                                yt[:, o, :], tmp[:], 0.01, tmp[:], MULT, MAX)
                sbase += len(pf)
                if f == 0:
                    # fp8 warm group for the small stage-C/D weights: their
                    # DMAs have landed by now; teaches the PE their ticks
                    warm2 = psg.tile([1, 4], F32, tag="g", name="warm2_ps")
                    for i, t_ in enumerate(w2qp):
                        nc.tensor.matmul(warm2[:], t_[:, 0:1, 0:1],
                                         t_[:, 0:1, 0:4],
                                         start=(i == 0), stop=False)
                    for i, t_ in enumerate(w2qs):
                        nc.tensor.matmul(warm2[:], t_[:, 0:1], t_[:, 0:4],
                                         start=False, stop=(i == 2))
                psc = psg.tile([128, FW], F32, tag="g", name=f"psc{f}")
                for p in range(2):
                    nc.tensor.matmul(psc[:], w1qt[p][:],
                                     yt[:, 2 * p:2 * p + 2, :],
                                     start=(p == 0), stop=(p == 1),
                                     perf_mode=DR)
                tmp3 = scr.tile([128, FW], F32, tag="t3", name=f"t3_{f}")
                nc.vector.tensor_scalar(tmp3[:], psc[:], cc, b1t[:],
                                        MULT, ADD)
                nc.vector.scalar_tensor_tensor(
                    x3[:, f * FW:(f + 1) * FW], tmp3[:], 0.01, tmp3[:],
                    MULT, MAX)
                # pad copies issue as soon as their x3 d-rows are complete,
                # so the DVE queue is clear when the conv phase starts
                while (next_pad < NDCH
                       and (next_pad * DCH + DCH) * TW <= (f + 1) * FW):
                    d0 = next_pad * DCH
                    nd = min(DCH, D - d0)
                    nc.vector.tensor_copy(
                        pad[:, 1 + d0:1 + d0 + nd, 1:TW + 1],
                        x3g[:, d0:d0 + nd, :])
                    next_pad += 1

            # ---- stage D: 3x3 conv over (d, t') with zero padding ----
            # dy 0+1 tap pairs via DoubleRow (overlapping strided AP), dy 2
            # as plain fp8 matmuls
            while next_pad < NDCH:
                d0 = next_pad * DCH
                nd = min(DCH, D - d0)
                nc.vector.tensor_copy(
                    pad[:, 1 + d0:1 + d0 + nd, 1:TW + 1],
                    x3g[:, d0:d0 + nd, :])
                next_pad += 1
            out_sb = outp.tile([1, D * TW], F32, tag="os", name="out_sb")
            x4cs = [None] * NDCH

            def conv_rhs_pair(d0, nd, dx):
                ap = pad[:, d0:d0 + 2, dx:dx + TW].unsqueeze(2).broadcast_to(
                    [128, 2, nd, TW]).copy()
                ap.ap[2] = [TWP, nd]
                return ap

            def stage_e(dc):
                d0 = dc * DCH
                fw = min(DCH, D - d0) * TW
                pse = psg.tile([1, DCH * TW], F32, tag="g", name=f"pse{dc}")
                nc.tensor.matmul(pse[:, 0:fw], w3t[:], x4cs[dc][:, 0:fw],
                                 start=True, stop=True)
                nc.scalar.activation(
                    out_sb[:, d0 * TW:d0 * TW + fw], pse[:, 0:fw],
                    mybir.ActivationFunctionType.Sigmoid,
                    bias=b3t[:], scale=1.0,
                )

            for dc in range(NDCH):
                d0 = dc * DCH
                nd = min(DCH, D - d0)
                fw = nd * TW
                psd = psg.tile([128, DCH * TW], F32, tag="d", name=f"psd{dc}")
                for dx in range(3):
                    nc.tensor.matmul(
                        psd[:, 0:fw],
                        w2qp[dx][:],
                        conv_rhs_pair(d0, nd, dx),
                        start=(dx == 0), stop=False,
                        perf_mode=DR,
                    )
                for dx in range(3):
                    nc.tensor.matmul(
                        psd[:, 0:fw],
                        w2qs[dx][:],
                        pad[:, d0 + 2:d0 + 2 + nd, dx:dx + TW],
                        start=False, stop=(dx == 2),
                    )
                x4c = x4p.tile([128, DCH * TW], BF16, tag="x4",
                               name=f"x4_{dc}")
                tmp4 = scr.tile([128, DCH * TW], F32, tag="t4",
                                name=f"t4_{dc}")
                nc.vector.tensor_scalar(tmp4[:, 0:fw], psd[:, 0:fw], cd,
                                        b2t[:], MULT, ADD)
                nc.vector.scalar_tensor_tensor(
                    x4c[:, 0:fw], tmp4[:, 0:fw], 0.01, tmp4[:, 0:fw],
                    MULT, MAX)
                x4cs[dc] = x4c
                # software pipeline: E for the previous chunk runs after the
                # next conv group is queued, hiding the ACT eviction latency
                if dc >= 1:
                    stage_e(dc - 1)
                if dc == 9:
                    # first-half output DMA on the idle SP ring overlaps the
                    # conv tail
                    nc.sync.dma_start(out_d[:, 0:9 * DCH * TW],
                                      out_sb[:, 0:9 * DCH * TW])
            stage_e(NDCH - 1)
            nc.sync.dma_start(out_d[:, 9 * DCH * TW:],
                              out_sb[:, 9 * DCH * TW:])
    _legalize_waits(nc)
    return nc


_PROGRAM = None


def _get_program(pairs):
    global _PROGRAM
    if _PROGRAM is None or _PROGRAM[0] != pairs:
        _PROGRAM = (pairs, _build_program(pairs))
    return _PROGRAM[1]


def _prep_inputs(feature, smp_weight, w0, b0, w1, b1, w2, b2, w3, b3):
    feature = np.ascontiguousarray(np.asarray(feature, dtype=np.float32))
    smp = np.asarray(smp_weight, dtype=np.float32).reshape(T, N, D, T)
    w0 = np.asarray(w0, dtype=np.float32)
    w1 = np.asarray(w1, dtype=np.float32)
    w2 = np.asarray(w2, dtype=np.float32)
    w3 = np.asarray(w3, dtype=np.float32)
    b0 = np.asarray(b0, dtype=np.float32)
    b1 = np.asarray(b1, dtype=np.float32)
    b2 = np.asarray(b2, dtype=np.float32)
    b3p = np.asarray(b3, dtype=np.float32).reshape(1, 1)

    # W slices per t-half: columns t' in [t0-1, t0+129), zero-padded outside
    # [0, T). Row-major layout (n, tau).
    wrows = []
    for th in range(2):
        t0 = th * 128
        lo, hi = t0 - 1, t0 + TW - 1
        clo, chi = max(lo, 0), min(hi, T)
        sl = np.zeros((T, N, D, TW), dtype=np.float32)
        sl[:, :, :, clo - lo:clo - lo + (chi - clo)] = smp[:, :, :, clo:chi]
        wrows.append(sl.transpose(1, 0, 2, 3).reshape(N * T, COLS))
    sw = max(np.abs(wr).max() for wr in wrows) / Q8
    colsum_max = max(np.abs(wr).sum(axis=0).max() for wr in wrows)

    # keep pattern: union over both halves (single SPMD program), paired
    # for DoubleRow (each slot contracts two 128-row chunks)
    nz0 = wrows[0].reshape(K, 128, NF, FW)
    nz1 = wrows[1].reshape(K, 128, NF, FW)
    pairs = []
    for f in range(NF):
        ks = sorted(set(np.nonzero(
            (np.abs(nz0[:, :, f, :]).max(axis=(1, 2)) > 0) |
            (np.abs(nz1[:, :, f, :]).max(axis=(1, 2)) > 0))[0].tolist()))
        if not ks:
            ks = [0]
        pf = []
        for i in range(0, len(ks) - 1, 2):
            pf.append((ks[i], ks[i + 1]))
        if len(ks) % 2:
            k = ks[-1]
            dummy = k - 1 if k > 0 else k + 1
            pf.append((min(k, dummy), max(k, dummy)))
        pairs.append(tuple(pf))
    pairs = tuple(pairs)

    # fp8 W pair tiles [slot, 128, 2, FW]; an odd slot's dummy half is zero
    wq = [np.asarray(wr / sw, dtype=E4NP) for wr in wrows]
    stot = sum(len(p) for p in pairs)
    wpk = [np.zeros((stot, 128, 2, FW), dtype=E4NP) for _ in range(2)]
    for th in range(2):
        nzs = (np.abs(wrows[th].reshape(K, 128, NF, FW)).max(axis=(1, 3)) > 0)
        s = 0
        for f in range(NF):
            for (ka, kb) in pairs[f]:
                for h, k in ((0, ka), (1, kb)):
                    if nzs[k, f]:
                        wpk[th][s, :, h, :] = \
                            wq[th][k * 128:(k + 1) * 128,
                                   f * FW:(f + 1) * FW]
                s += 1

    # fp8 packing of w1 (o-chunk pairs) and w2 (dy 0+1 pairs + dy 2 singles)
    sw1 = np.abs(w1).max() / Q8
    w1T = (w1.T / sw1)                                        # (512, 128)
    w1q = np.zeros((2, 128, 2, DIM1), dtype=E4NP)
    for p in range(2):
        for h in range(2):
            w1q[p, :, h, :] = w1T[(2 * p + h) * 128:(2 * p + h + 1) * 128]
    sw2 = np.abs(w2).max() / Q8
    w2j = w2.transpose(2, 3, 1, 0) / sw2                      # (dy, dx, c, o)
    w2qp = np.zeros((3, 128, 2, DIM1), dtype=E4NP)
    for dx in range(3):
        for h in range(2):
            w2qp[dx, :, h, :] = w2j[h, dx]
    w2qs = np.ascontiguousarray(np.asarray(w2j[2], dtype=E4NP))  # (3, c, o)

    # fp8 packing of feat (c-chunk pairs) and w0; per-batch fp8 scales
    # folded into the ACT scale/bias inputs
    sw0 = np.abs(w0).max() / Q8
    w08 = np.ascontiguousarray(np.asarray(
        (w0 / sw0).transpose(2, 1, 0).reshape(N, 2, 128, DIM0)
        .transpose(0, 2, 1, 3), dtype=E4NP))        # (N, 128, 2, DIM0)
    A = np.einsum('bct,ocn->bnto', feature, w0, optimize=True)
    w1_rowsum = np.abs(w1).sum(axis=1).max()
    per_b = []
    for b in range(B):
        sf = np.abs(feature[b]).max() / Q8
        amax = np.abs(A[b]).max()
        # 5% headroom: the device A is computed from fp8 inputs and can
        # slightly exceed the host fp32 max; e4m3 overflows past 240
        sa_r = amax * 1.05 / (sf * sw0 * Q8)
        sa = sf * sw0 * sa_r
        x2bound = amax * colsum_max + np.abs(b0).max() + 1e-6
        sx2 = x2bound / Q8
        x3bound = w1_rowsum * x2bound + np.abs(b1).max() + 1e-6
        sx3 = x3bound / Q8
        scl = np.empty((4, 128, 1), np.float32)
        scl[0] = sa * sw / sx2          # cx
        scl[1] = sw1 * sx2 / sx3        # cc
        scl[2] = sw2 * sx3              # cd (un-scales fully; x4 is true)
        scl[3] = 1.0 / sa_r             # ca (stage A psum -> fp8 a8)
        per_b.append(dict(
            feat=np.ascontiguousarray(np.asarray(
                (feature[b] / sf).reshape(2, 128, T).transpose(1, 0, 2),
                dtype=E4NP)),                        # (128, 2, T)
            b0x=np.ascontiguousarray((b0 / sx2).reshape(4, 128, 1)),
            b1c=(b1 / sx3).reshape(128, 1),
            scl=scl,
        ))
    w3q = np.ascontiguousarray(np.asarray(w3.T, dtype=BFNP))  # (128, 1)
    b2d = b2.reshape(128, 1)
    return (wpk, pairs, per_b, w08, w1q, w2qp, w2qs, w3q, b2d, b3p)


def kernel(feature, smp_weight, w0, b0, w1, b1, w2, b2, w3, b3,
           _trace=False):
    (wpk, pairs, per_b, w08, w1q, w2qp, w2qs, w3q, b2d,
     b3p) = _prep_inputs(
        feature, smp_weight, w0, b0, w1, b1, w2, b2, w3, b3)

    nc = _get_program(pairs)
    in_maps = []
    for core in range(8):
        b, th = core // 2, core % 2
        in_maps.append({
            "feat": per_b[b]["feat"],
            "w0t": w08,
            "wsmp": wpk[th],
            "w1q": w1q,
            "w2qp": w2qp,
            "w2qs": w2qs,
            "w3t": w3q,
            "b0x": per_b[b]["b0x"],
            "b1c": per_b[b]["b1c"],
            "b2d": b2d,
            "b3": b3p,
            "scl": per_b[b]["scl"],
        })
    res = run_bass_kernel_spmd(nc, in_maps, core_ids=list(range(8)),
                               trace=_trace)
    out = np.empty((B, D, T), dtype=np.float32)
    for core in range(8):
        b, th = core // 2, core % 2
        full = res.results[core]["out"].reshape(D, TW)
        out[b, :, th * 128:(th + 1) * 128] = full[:, 1:TW - 1]
    if _trace:
        return out, res
    return out
